# revision 32
# baseline (speedup 1.0000x reference)
"""Causal RoPE self-attention, distributed over 8 TRN2 NeuronCores.

Sharding: batch (2) x head-groups (4 heads each) -> 8 cores.
Each core computes, for its (batch b, head-group hg):
    q/k/v projections for its 4 heads (tensor-parallel column split),
    RoPE, causal attention, and the row-parallel slice of the output
    projection, producing a partial output partialT = WoS^T @ attnT
    of shape [E, S].  The host sums the 4 partials per batch and adds bo.

On-device layout notes:
  - activations live transposed: qT/kT are [head-dim, seq] so the
    score matmul sT[k, q] = K Q^T contracts over d on partitions (the
    two heads of a pair row-tile the PE array at K=64 each), and the
    softmax denominator comes from an extra all-ones column in V.
  - x, all weights, qT/kT, exp'd scores, and V are bf16 (full-rate
    TensorEngine, FWL weight loads, half DMA); every accumulation is
    fp32 in PSUM, and the softmax/normalization math is fp32.
  - input DMA descriptor issue is the startup bottleneck (~600ns per
    dma_start on an engine queue): wk/wq/wv ship as ONE concatenated
    host tensor (8 descriptors), inputs split across the sync AND
    scalar (Activation) HWDGE queues, and x arrives in 512-col
    quarters for the first S-half so the first projections start
    after ~1MB instead of ~4MB.
  - kT/qT are built as [128, 512] half-tiles so the first attention
    phase needs only keys/queries 0-511; k00a/q00a are projected
    during the DMA window and the attention stream starts right after
    their RoPE, with everything else (k10a/q10a, V tiles, later
    projections, output projection) drip-fed as background units.
    Per-block "ensure" flushes guarantee each block's kt/qt/v
    producers are emitted before their consumers.
  - causal masking zeroes the exp'd diagonal blocks with a gpsimd
    affine_select; gpsimd runs ONLY affine_selects so the exp->mask->AV
    chain never queues behind another engine's work.
  - attention PSUM accumulators are evicted to SBUF immediately after
    the last key-block; the phase-end DVE chain is just
    evict+den+reciprocal. The denominator broadcast runs on the
    TensorEngine (col-tiled ones x rec outer products into PSUM) and
    the final normalization muls are a background unit, so the next
    phase's exp stream never waits on them.
  - the output projection for the last q-slice is split into p0/p1
    half-units staged through SBUF; its final DMAs alternate between
    the sync and scalar queues to shorten the tail.
"""

import ml_dtypes
import numpy as np

import concourse.tile as tile
from concourse import bacc, mybir
from concourse.bass_utils import run_bass_kernel_spmd

F32 = mybir.dt.float32
BF16 = mybir.dt.bfloat16
AF = mybir.ActivationFunctionType

B, S, E = 2, 2048, 1024
H, D = 16, 64
HPG = 4                # heads per core
DH = HPG * D           # 256 head-dims per core
NE = E // 128          # 8 e-chunks
NST = S // 128         # 16 s-tiles / key blocks
ROPE_BASE = 10000.0

_SWAP_MASK = [i ^ 1 for i in range(32)]
# column offsets of wk / wq / wv inside the concatenated wkqv tile
WK0, WQ0, WV0 = 0, DH, 2 * DH


def build_nc():
    """Build + compile the per-core Bass graph (same graph on all 8 cores)."""
    nc = bacc.Bacc("TRN2", target_bir_lowering=False, debug=False, num_devices=8)

    def din(name, shape, dt=F32):
        return nc.dram_tensor(name, shape, dt, kind="ExternalInput").ap()

    xT = din("xT", [E, S], BF16)
    wkq = din("wkq", [E, 2 * DH], BF16)
    wvT = din("wvT", [E, DH], BF16)
    woST = din("woST", [DH, E], BF16)
    bq2 = din("bq2", [128, 2])
    bk2 = din("bk2", [128, 2])
    bvbc = din("bvbc", [128, DH])
    cos2 = din("cos2", [128, S], BF16)      # cosT duplicated on both halves
    sin2 = din("sin2", [128, S], BF16)      # signed sinT duplicated on both halves
    out = nc.dram_tensor("out", [E, S], BF16, kind="ExternalOutput").ap()

    xT_r = xT.rearrange("(n p) s -> n p s", p=128)
    wkq_r = wkq.rearrange("(n p) d -> n p d", p=128)
    wv_r = wvT.rearrange("(n p) d -> n p d", p=128)
    wo_r = woST.rearrange("(n p) e -> n p e", p=128)
    out_r = out.rearrange("(n p) s -> n p s", p=128)

    with tile.TileContext(nc) as tc, nc.allow_low_precision(
            reason="bf16 matmul operands; fp32 PSUM accumulation throughout"):
        _emit(tc, nc, dict(
            xT_r=xT_r, wkq_r=wkq_r, wv_r=wv_r, wo_r=wo_r, out_r=out_r,
            bq2=bq2, bk2=bk2, bvbc=bvbc, cos2=cos2, sin2=sin2,
        ))
    nc.compile()
    return nc


def _emit(tc, nc, d):
    from contextlib import ExitStack
    ctx = ExitStack()
    with ctx:
        consts = ctx.enter_context(tc.tile_pool(name="consts", bufs=1))
        pxq = ctx.enter_context(tc.tile_pool(name="pxq", bufs=16))
        pxh = ctx.enter_context(tc.tile_pool(name="pxh", bufs=8))
        pw = ctx.enter_context(tc.tile_pool(name="pw", bufs=8))
        pwv = ctx.enter_context(tc.tile_pool(name="pwv", bufs=8))
        pwo = ctx.enter_context(tc.tile_pool(name="pwo", bufs=2))
        pqt = ctx.enter_context(tc.tile_pool(name="pqt", bufs=8))
        pkt = ctx.enter_context(tc.tile_pool(name="pkt", bufs=8))
        pv = ctx.enter_context(tc.tile_pool(name="pv", bufs=16))
        pat = ctx.enter_context(tc.tile_pool(name="pat", bufs=8))
        ptmp = ctx.enter_context(tc.tile_pool(name="ptmp", bufs=6))
        pvf_ = ctx.enter_context(tc.tile_pool(name="pvf", bufs=4))
        pbc = ctx.enter_context(tc.tile_pool(name="pbc", bufs=4))
        pstg = ctx.enter_context(tc.tile_pool(name="pstg", bufs=8))
        pe_ = ctx.enter_context(tc.tile_pool(name="pe", bufs=10))
        prec = ctx.enter_context(tc.tile_pool(name="prec", bufs=4))
        psc = ctx.enter_context(tc.tile_pool(name="psc", bufs=2, space="PSUM"))
        ppv = ctx.enter_context(tc.tile_pool(name="ppv", bufs=2, space="PSUM"))
        pbg = ctx.enter_context(tc.tile_pool(name="pbg", bufs=2, space="PSUM"))

        # ---- input DMAs, split across the sync and scalar HWDGE queues.
        # scalar (idle until the first exp) carries ONLY the chunk-loop
        # prefix (wk|wq) + rope tables; sync carries x and everything else.
        # Order within each queue = need order.
        w_sb, wv_sb, wo_sb = {}, {}, []
        xq_sb, xh_sb = {}, {}

        for e in range(NE):
            t = pw.tile([128, 2 * DH], BF16, tag="w")
            nc.scalar.dma_start(t, d["wkq_r"][e])
            w_sb[e] = t
        cos2_sb = consts.tile([128, S], BF16)
        sin2_sb = consts.tile([128, S], BF16)
        nc.scalar.dma_start(cos2_sb[:, 0:1024], d["cos2"][:, 0:1024])
        nc.scalar.dma_start(sin2_sb[:, 0:1024], d["sin2"][:, 0:1024])

        for e in range(NE):
            t = pxq.tile([128, 512], BF16, tag="xq")
            nc.sync.dma_start(t, d["xT_r"][e][:, 0:512])
            xq_sb[(e, 0)] = t
        bq2_sb = consts.tile([128, 2], F32)
        nc.sync.dma_start(bq2_sb, d["bq2"])
        bk2_sb = consts.tile([128, 2], F32)
        nc.sync.dma_start(bk2_sb, d["bk2"])
        for e in range(NE):
            t = pwv.tile([128, DH], BF16, tag="wv")
            nc.sync.dma_start(t, d["wv_r"][e])
            wv_sb[e] = t
        bvbc_sb = consts.tile([128, DH], F32)
        nc.sync.dma_start(bvbc_sb, d["bvbc"])
        for e in range(NE):
            t = pxq.tile([128, 512], BF16, tag="xq")
            nc.sync.dma_start(t, d["xT_r"][e][:, 512:1024])
            xq_sb[(e, 1)] = t
        nc.sync.dma_start(cos2_sb[:, 1024:2048], d["cos2"][:, 1024:2048])
        nc.sync.dma_start(sin2_sb[:, 1024:2048], d["sin2"][:, 1024:2048])
        for e in range(NE):
            t = pxh.tile([128, 1024], BF16, tag="xh")
            nc.sync.dma_start(t, d["xT_r"][e][:, 1024:2048])
            xh_sb[e] = t
        for p in range(2):
            t = pwo.tile([128, E], BF16, tag="wo")
            nc.sync.dma_start(t, d["wo_r"][p])
            wo_sb.append(t)

        warm_sb = consts.tile([128, 512], BF16)
        nc.vector.memset(warm_sb, 0.25)

        def xs(e, scol, w=512):
            """SBUF view of x columns [scol, scol+w) for e-chunk e."""
            if scol < 1024:
                q, off = divmod(scol, 512)
                return xq_sb[(e, q)][:, off:off + w]
            off = scol - 1024
            return xh_sb[e][:, off:off + w]

        # ---- emission: a fine-grained interleave. The PE is the busiest
        # engine mid-kernel; the softmax exps on the scalar engine pace the
        # attention stream. All non-attention PE work is drip-fed between
        # key-blocks; per-block ensures flush producers just in time.
        qt_tiles, kt_tiles, at_tiles = {}, {}, {}
        v_sb = {}
        op_stage = {}

        def rope_tail(ps, bias_sb, dst_pool, dst_tag, tiles, p, idx):
            """Evict a [128,512] qk PSUM accumulator and apply RoPE."""
            cols = slice(idx * 512, idx * 512 + 512)
            tq = ptmp.tile([128, 512], BF16, tag="tmpb")
            nc.vector.tensor_scalar_add(tq, ps, bias_sb[:, p:p + 1])
            tsh = ptmp.tile([128, 512], BF16, tag="tmpb")
            nc.vector.stream_shuffle(tsh, tq, _SWAP_MASK)
            nc.vector.tensor_mul(tsh, tsh, sin2_sb[:, cols])
            nc.vector.tensor_mul(tq, tq, cos2_sb[:, cols])
            qt = dst_pool.tile([128, 512], BF16, tag=dst_tag)
            nc.vector.tensor_add(qt, tq, tsh)
            tiles[(p, idx)] = qt

        def emit_qk_half(w_off, bias_sb, dst_pool, dst_tag, tiles, p, idx):
            """One [128,512] k or q half-tile: 8 e-chunk matmuls + RoPE.
            Yields background units (per e-chunk, then the tail)."""
            ps = pbg.tile([128, 512], F32, tag="bg")
            for e in range(NE):
                def unit(e=e):
                    nc.tensor.matmul(
                        ps,
                        w_sb[e][:, w_off + p * 128:w_off + (p + 1) * 128],
                        xs(e, idx * 512),
                        start=(e == 0), stop=(e == NE - 1),
                    )
                yield 0.22, unit
            yield 0.1, lambda: rope_tail(
                ps, bias_sb, dst_pool, dst_tag, tiles, p, idx)

        def emit_v_unit(st):
            def unit():
                psv = pbg.tile([128, DH], F32, tag="bg")
                for e in range(NE):
                    nc.tensor.matmul(
                        psv,
                        xs(e, st * 128, 128),
                        wv_sb[e],
                        start=(e == 0), stop=(e == NE - 1),
                    )
                vt = pv.tile([128, HPG, 65], BF16, tag="v")
                nc.vector.memset(vt[:, :, 64:65], 1.0)
                nc.vector.tensor_add(
                    vt[:, :, 0:64],
                    psv.rearrange("p (h dd) -> p h dd", dd=64),
                    bvbc_sb.rearrange("p (h dd) -> p h dd", dd=64),
                )
                v_sb[st] = vt
            yield 1.1, unit

        def emit_at_unit(p, j, pvf, rec):
            """Normalize a finished phase: gpsimd-broadcast 1/den across
            partitions, then the two DVE muls. Runs as background (at least
            one phase later) so the reciprocal is long since ready and the
            next phase's exp/affine stream never waits on it."""
            def unit():
                bcsA = pbc.tile([64, 512], F32, tag="bc")
                bcsB = pbc.tile([64, 512], F32, tag="bc")
                nc.gpsimd.partition_broadcast(bcsA, rec[:, 0:512])
                nc.gpsimd.partition_broadcast(bcsB, rec[:, 512:1024])
                at = pat.tile([128, 512], BF16, tag="at")
                nc.vector.tensor_mul(at[0:64], pvf[0:64, 0:512], bcsA)
                nc.vector.tensor_mul(at[64:128], pvf[0:64, 512:1024], bcsB)
                at_tiles[(p, j)] = at
            yield 0.5, unit

        def emit_op_unit(j):
            """Fused output-projection unit: both head-pairs accumulate in
            PSUM, one bf16 staging copy, one DMA."""
            for et_i in range(NE):
                def unit(et_i=et_i):
                    pso = pbg.tile([128, 512], F32, tag="bg")
                    for p in range(2):
                        nc.tensor.matmul(
                            pso,
                            wo_sb[p][:, et_i * 128:(et_i + 1) * 128],
                            at_tiles[(p, j)],
                            start=(p == 0), stop=(p == 1),
                        )
                    ob = pstg.tile([128, 512], BF16, tag="ob")
                    nc.vector.tensor_copy(ob, pso)
                    nc.sync.dma_start(
                        d["out_r"][et_i][:, j * 512:(j + 1) * 512], ob)
                yield 0.7, unit

        def emit_op_p0(j):
            """First head-pair's half of the output projection for q-slice j;
            accumulates into an SBUF stage so it can run as soon as at(0,j)
            exists, one attention phase before at(1,j). Used for the final
            q-slice only, to shorten the tail."""
            for et_i in range(NE):
                def unit(et_i=et_i):
                    pso = pbg.tile([128, 512], F32, tag="bg")
                    nc.tensor.matmul(
                        pso,
                        wo_sb[0][:, et_i * 128:(et_i + 1) * 128],
                        at_tiles[(0, j)],
                        start=True, stop=True,
                    )
                    stg = pstg.tile([128, 512], F32, tag="stg")
                    nc.vector.tensor_copy(stg, pso)
                    op_stage[(j, et_i)] = stg
                yield 0.4, unit

        def emit_op_p1(j):
            for et_i in range(NE):
                def unit(et_i=et_i):
                    pso = pbg.tile([128, 512], F32, tag="bg")
                    nc.tensor.matmul(
                        pso,
                        wo_sb[1][:, et_i * 128:(et_i + 1) * 128],
                        at_tiles[(1, j)],
                        start=True, stop=True,
                    )
                    ob = pstg.tile([128, 512], BF16, tag="ob")
                    nc.vector.tensor_add(ob, op_stage[(j, et_i)], pso)
                    eng = nc.scalar if et_i % 2 == 0 else nc.sync
                    eng.dma_start(
                        d["out_r"][et_i][:, j * 512:(j + 1) * 512], ob)
                yield 0.45, unit

        # background unit queue + driver, with named completion points
        bg_units = []
        bg_pos = [0]
        kt_ready, qt_ready, v_ready = {}, {}, {}

        def bg_add(gen):
            bg_units.extend(gen)
            return len(bg_units)

        def bg_flush_until(idx):
            while bg_pos[0] < idx:
                bg_units[bg_pos[0]][1]()
                bg_pos[0] += 1

        def bg_take(budget):
            while budget > 0 and bg_pos[0] < len(bg_units):
                cost, fn = bg_units[bg_pos[0]]
                fn()
                bg_pos[0] += 1
                budget -= cost

        def emit_attn(p, j, take=0.55):
            if (p, j) in qt_ready:
                bg_flush_until(qt_ready[(p, j)])
            pvA = ppv.tile([128, 512], F32, tag="ppv")
            pvB = ppv.tile([128, 512], F32, tag="ppv")
            nkb = 4 * j + 4
            for kb in range(nkb):
                s4 = kb // 4
                if (p, s4) in kt_ready:
                    bg_flush_until(kt_ready[(p, s4)])
                m = kb - 4 * j
                c0 = 128 * m if m > 0 else 0
                kt = kt_tiles[(p, s4)]
                kcols = slice((kb % 4) * 128, (kb % 4) * 128 + 128)
                qt = qt_tiles[(p, j)]
                qcols = slice(c0, 512)
                sc = psc.tile([128, 2, 512], F32, tag="sc")
                nc.tensor.matmul(
                    sc[:, 0, c0:512],
                    kt[0:64, kcols],
                    qt[0:64, qcols],
                    start=True, stop=True, tile_position=(0, 0),
                )
                nc.tensor.matmul(
                    sc[:, 1, c0:512],
                    kt[64:128, kcols],
                    qt[64:128, qcols],
                    start=True, stop=True, tile_position=(64, 0),
                )
                et = pe_.tile([128, 2, 512], BF16, tag="e")
                nc.scalar.activation(
                    et[:, :, c0:512], sc[:, :, c0:512], AF.Exp, scale=0.125)
                # ensure this block's V tile producers are emitted before
                # its AV consumers (the flush runs while the exp executes)
                if kb in v_ready:
                    bg_flush_until(v_ready[kb])
                if m >= 0:
                    nc.gpsimd.affine_select(
                        out=et[:, :, c0:c0 + 128],
                        in_=et[:, :, c0:c0 + 128],
                        compare_op=mybir.AluOpType.is_ge,
                        fill=0.0,
                        base=0,
                        pattern=[[0, 2], [1, 128]],
                        channel_multiplier=-1,
                    )
                hA, hB = 2 * p, 2 * p + 1
                nc.tensor.matmul(
                    pvA[0:65, c0:512], v_sb[kb][:, hA, :], et[:, 0, c0:512],
                    start=(kb == 0), stop=(kb == nkb - 1),
                )
                nc.tensor.matmul(
                    pvB[0:65, c0:512], v_sb[kb][:, hB, :], et[:, 1, c0:512],
                    start=(kb == 0), stop=(kb == nkb - 1),
                )
                bg_take(take)
            # Evict the PSUM accumulators to SBUF right away: frees both ppv
            # banks for the next phase's AV matmuls. Phase-end DVE chain is
            # just evict + den + reciprocal; normalization muls run later as
            # a background at-unit.
            pvf = pvf_.tile([128, 1024], F32, tag="pvf")
            nc.vector.tensor_copy(pvf[0:65, 0:512], pvA[0:65, :])
            nc.vector.tensor_copy(pvf[0:65, 512:1024], pvB[0:65, :])
            # den must be a base-0 AP: custom-DVE ops (reciprocal) ignore the
            # input's base partition, so slice pvf[64:65] via a copy first.
            den = prec.tile([1, 1024], F32, tag="den")
            nc.vector.tensor_copy(den, pvf[64:65, :])
            rec = prec.tile([1, 1024], F32, tag="rec")
            nc.vector.reciprocal_approx_fast(rec, den)
            return pvf, rec

        # ---- schedule ----
        # Warm-up: ~3.5us of dummy matmuls on a memset const during the DMA
        # window, so the HAM clock gate reaches K=8/8 (2.4 GHz) before the
        # first real projection instead of running the whole chunk loop at
        # the cold 1.2 GHz default.
        ps_warm = pbg.tile([128, 512], F32, tag="bg")
        for _ in range(10):
            nc.tensor.matmul(ps_warm, warm_sb[:, 0:128], warm_sb,
                             start=True, stop=True)
        # Startup: all four first half-tiles (k00a/q00a/k10a/q10a)
        # interleaved per e-chunk (tracks the xQ0 DMA stream), then RoPE
        # chains on the DVE; v0-3 ensured inside the first phase.
        ps_k00 = pbg.tile([128, 512], F32, tag="bg")
        ps_q00 = pbg.tile([128, 512], F32, tag="bg")
        ps_k10 = ppv.tile([128, 512], F32, tag="ppv")
        ps_q10 = ppv.tile([128, 512], F32, tag="ppv")
        for e in range(NE):
            nc.tensor.matmul(ps_k00, w_sb[e][:, WK0:WK0 + 128], xs(e, 0),
                             start=(e == 0), stop=(e == NE - 1))
            nc.tensor.matmul(ps_q00, w_sb[e][:, WQ0:WQ0 + 128], xs(e, 0),
                             start=(e == 0), stop=(e == NE - 1))
            nc.tensor.matmul(ps_k10, w_sb[e][:, WK0 + 128:WK0 + 256], xs(e, 0),
                             start=(e == 0), stop=(e == NE - 1))
            nc.tensor.matmul(ps_q10, w_sb[e][:, WQ0 + 128:WQ0 + 256], xs(e, 0),
                             start=(e == 0), stop=(e == NE - 1))
        rope_tail(ps_k00, bk2_sb, pkt, "kt", kt_tiles, 0, 0)
        rope_tail(ps_q00, bq2_sb, pqt, "qt", qt_tiles, 0, 0)
        rope_tail(ps_k10, bk2_sb, pkt, "kt", kt_tiles, 1, 0)
        rope_tail(ps_q10, bq2_sb, pqt, "qt", qt_tiles, 1, 0)
        v_ready[0] = bg_add(emit_v_unit(0))
        v_ready[1] = bg_add(emit_v_unit(1))
        v_ready[2] = bg_add(emit_v_unit(2))
        v_ready[3] = bg_add(emit_v_unit(3))

        qt_ready[(0, 1)] = bg_add(
            emit_qk_half(WQ0, bq2_sb, pqt, "qt", qt_tiles, 0, 1))
        kt_ready[(0, 1)] = bg_add(
            emit_qk_half(WK0, bk2_sb, pkt, "kt", kt_tiles, 0, 1))
        v_ready[4] = bg_add(emit_v_unit(4))
        v_ready[5] = bg_add(emit_v_unit(5))
        v_ready[6] = bg_add(emit_v_unit(6))
        v_ready[7] = bg_add(emit_v_unit(7))
        qt_ready[(1, 1)] = bg_add(
            emit_qk_half(WQ0, bq2_sb, pqt, "qt", qt_tiles, 1, 1))
        kt_ready[(1, 1)] = bg_add(
            emit_qk_half(WK0, bk2_sb, pkt, "kt", kt_tiles, 1, 1))

        pvf00, rec00 = emit_attn(0, 0)
        bg_add(emit_at_unit(0, 0, pvf00, rec00))
        pvf10, rec10 = emit_attn(1, 0)
        bg_add(emit_at_unit(1, 0, pvf10, rec10))
        bg_add(emit_op_unit(0))
        pvf01, rec01 = emit_attn(0, 1)
        bg_add(emit_at_unit(0, 1, pvf01, rec01))
        qt_ready[(0, 2)] = bg_add(
            emit_qk_half(WQ0, bq2_sb, pqt, "qt", qt_tiles, 0, 2))
        kt_ready[(0, 2)] = bg_add(
            emit_qk_half(WK0, bk2_sb, pkt, "kt", kt_tiles, 0, 2))
        v_ready[8] = bg_add(emit_v_unit(8))
        v_ready[9] = bg_add(emit_v_unit(9))
        v_ready[10] = bg_add(emit_v_unit(10))
        v_ready[11] = bg_add(emit_v_unit(11))
        qt_ready[(1, 2)] = bg_add(
            emit_qk_half(WQ0, bq2_sb, pqt, "qt", qt_tiles, 1, 2))
        kt_ready[(1, 2)] = bg_add(
            emit_qk_half(WK0, bk2_sb, pkt, "kt", kt_tiles, 1, 2))
        pvf11, rec11 = emit_attn(1, 1)
        bg_add(emit_at_unit(1, 1, pvf11, rec11))
        bg_add(emit_op_unit(1))
        qt_ready[(0, 3)] = bg_add(
            emit_qk_half(WQ0, bq2_sb, pqt, "qt", qt_tiles, 0, 3))
        kt_ready[(0, 3)] = bg_add(
            emit_qk_half(WK0, bk2_sb, pkt, "kt", kt_tiles, 0, 3))
        v_ready[12] = bg_add(emit_v_unit(12))
        v_ready[13] = bg_add(emit_v_unit(13))
        v_ready[14] = bg_add(emit_v_unit(14))
        v_ready[15] = bg_add(emit_v_unit(15))
        pvf02, rec02 = emit_attn(0, 2)
        bg_add(emit_at_unit(0, 2, pvf02, rec02))
        qt_ready[(1, 3)] = bg_add(
            emit_qk_half(WQ0, bq2_sb, pqt, "qt", qt_tiles, 1, 3))
        kt_ready[(1, 3)] = bg_add(
            emit_qk_half(WK0, bk2_sb, pkt, "kt", kt_tiles, 1, 3))
        pvf12, rec12 = emit_attn(1, 2)
        bg_add(emit_at_unit(1, 2, pvf12, rec12))
        bg_add(emit_op_unit(2))
        pvf03, rec03 = emit_attn(0, 3, take=0.7)
        bg_add(emit_at_unit(0, 3, pvf03, rec03))
        bg_add(emit_op_p0(3))
        pvf13, rec13 = emit_attn(1, 3, take=0.7)
        bg_flush_until(len(bg_units))
        # keep the PE warm (and the HAM un-throttled) across the final
        # normalization chain so the last output-projection matmuls run at
        # full clock
        ps_warm2 = pbg.tile([128, 512], F32, tag="bg")
        for _ in range(14):
            nc.tensor.matmul(ps_warm2, warm_sb[:, 0:128], warm_sb,
                             start=True, stop=True)
        for cost, fn in emit_at_unit(1, 3, pvf13, rec13):
            fn()
        for cost, fn in emit_op_p1(3):
            fn()


def make_host_inputs(x, Wq, bq, Wk, bk, Wv, bv, Wo, bo):
    """Shard + pre-transpose inputs per core. Returns (in_maps, bo)."""
    x = np.asarray(x, np.float32)
    Wq, Wk, Wv, Wo = (np.asarray(w, np.float32) for w in (Wq, Wk, Wv, Wo))
    bq, bk, bv, bo = (np.asarray(b_, np.float32) for b_ in (bq, bk, bv, bo))

    # RoPE tables
    half = D // 2
    inv_freq = 1.0 / (ROPE_BASE ** (np.arange(half, dtype=np.float64) / half))
    pos = np.arange(S, dtype=np.float64)
    sinus = pos[:, None] * inv_freq[None, :]           # [S, 32]
    sin_full = np.repeat(np.sin(sinus), 2, axis=1)     # [S, 64] interleave-dup
    cos_full = np.repeat(np.cos(sinus), 2, axis=1)
    sgn = np.where(np.arange(D) % 2 == 0, -1.0, 1.0)
    cos2 = np.tile(cos_full.T, (2, 1)).astype(ml_dtypes.bfloat16)
    sin2 = np.tile((sin_full * sgn[None, :]).T, (2, 1)).astype(ml_dtypes.bfloat16)

    xT = [np.ascontiguousarray(x[b_].T) for b_ in range(B)]
    in_maps = []
    for c in range(8):
        b_, hg = c // 4, c % 4
        rows = slice(DH * hg, DH * hg + DH)
        bf = ml_dtypes.bfloat16
        wkq = np.concatenate([Wk[rows].T, Wq[rows].T], axis=1)  # [E, 2*DH]
        in_maps.append({
            "xT": xT[b_].astype(bf),
            "wkq": np.ascontiguousarray(wkq).astype(bf),
            "wvT": np.ascontiguousarray(Wv[rows].T).astype(bf),
            "woST": np.ascontiguousarray(Wo[:, rows].T).astype(bf),
            "bq2": np.ascontiguousarray(bq[rows].reshape(2, 128).T),
            "bk2": np.ascontiguousarray(bk[rows].reshape(2, 128).T),
            "bvbc": np.tile(bv[rows][None, :], (128, 1)).astype(np.float32),
            "cos2": cos2,
            "sin2": sin2,
        })
    return in_maps, bo


_NC_CACHE = {}


def get_nc():
    if "nc" not in _NC_CACHE:
        _NC_CACHE["nc"] = build_nc()
    return _NC_CACHE["nc"]


def kernel(**inputs):
    in_maps, bo = make_host_inputs(**inputs)
    nc = get_nc()
    res = run_bass_kernel_spmd(nc, in_maps, core_ids=list(range(8)))
    out = np.zeros((B, S, E), np.float32)
    for c in range(8):
        out[c // 4] += np.asarray(res.results[c]["out"], np.float32).T
    out += bo[None, None, :]
    return out


# revision 33
# speedup vs baseline: 1.0428x; 1.0428x over previous
"""Causal RoPE self-attention, distributed over 8 TRN2 NeuronCores.

Sharding: batch (2) x head-groups (4 heads each) -> 8 cores.
Each core computes, for its (batch b, head-group hg):
    q/k/v projections for its 4 heads (tensor-parallel column split),
    RoPE, causal attention, and the row-parallel slice of the output
    projection, producing a partial output partialT = WoS^T @ attnT
    of shape [E, S].  The host sums the 4 partials per batch and adds bo.

On-device layout notes:
  - activations live transposed: qT/kT are [head-dim, seq] so the
    score matmul sT[k, q] = K Q^T contracts over d on partitions (the
    two heads of a pair row-tile the PE array at K=64 each), and the
    softmax denominator comes from an extra all-ones column in V.
  - x, all weights, qT/kT, exp'd scores, and V are bf16 (full-rate
    TensorEngine, FWL weight loads, half DMA); every accumulation is
    fp32 in PSUM, and the softmax/normalization math is fp32.
  - input DMA descriptor issue is the startup bottleneck (~600ns per
    dma_start on an engine queue): wk/wq/wv ship as ONE concatenated
    host tensor (8 descriptors), inputs split across the sync AND
    scalar (Activation) HWDGE queues, and x arrives in 512-col
    quarters for the first S-half so the first projections start
    after ~1MB instead of ~4MB.
  - kT/qT are built as [128, 512] half-tiles so the first attention
    phase needs only keys/queries 0-511; k00a/q00a are projected
    during the DMA window and the attention stream starts right after
    their RoPE, with everything else (k10a/q10a, V tiles, later
    projections, output projection) drip-fed as background units.
    Per-block "ensure" flushes guarantee each block's kt/qt/v
    producers are emitted before their consumers.
  - causal masking zeroes the exp'd diagonal blocks with a gpsimd
    affine_select; gpsimd runs ONLY affine_selects so the exp->mask->AV
    chain never queues behind another engine's work.
  - attention PSUM accumulators are evicted to SBUF immediately after
    the last key-block; the phase-end DVE chain is just
    evict+den+reciprocal. The denominator broadcast runs on the
    TensorEngine (col-tiled ones x rec outer products into PSUM) and
    the final normalization muls are a background unit, so the next
    phase's exp stream never waits on them.
  - the output projection for the last q-slice is split into p0/p1
    half-units staged through SBUF; its final DMAs alternate between
    the sync and scalar queues to shorten the tail.
"""

import ml_dtypes
import numpy as np

import concourse.tile as tile
from concourse import bacc, mybir
from concourse.bass_utils import run_bass_kernel_spmd

F32 = mybir.dt.float32
BF16 = mybir.dt.bfloat16
AF = mybir.ActivationFunctionType

B, S, E = 2, 2048, 1024
H, D = 16, 64
HPG = 4                # heads per core
DH = HPG * D           # 256 head-dims per core
NE = E // 128          # 8 e-chunks
NST = S // 128         # 16 s-tiles / key blocks
ROPE_BASE = 10000.0

_SWAP_MASK = [i ^ 1 for i in range(32)]
# column offsets of wk / wq / wv inside the concatenated wkqv tile
WK0, WQ0, WV0 = 0, DH, 2 * DH


def build_nc():
    """Build + compile the per-core Bass graph (same graph on all 8 cores)."""
    nc = bacc.Bacc("TRN2", target_bir_lowering=False, debug=False, num_devices=8)

    def din(name, shape, dt=F32):
        return nc.dram_tensor(name, shape, dt, kind="ExternalInput").ap()

    xT = din("xT", [E, S], BF16)
    wkq = din("wkq", [E, 2 * DH], BF16)
    wvT = din("wvT", [E, DH], BF16)
    woST = din("woST", [DH, E], BF16)
    bq2 = din("bq2", [128, 2])
    bk2 = din("bk2", [128, 2])
    bvbc = din("bvbc", [128, DH])
    cos2 = din("cos2", [128, S], BF16)      # cosT duplicated on both halves
    sin2 = din("sin2", [128, S], BF16)      # signed sinT duplicated on both halves
    out = nc.dram_tensor("out", [E, S], BF16, kind="ExternalOutput").ap()

    xT_r = xT.rearrange("(n p) s -> n p s", p=128)
    wkq_r = wkq.rearrange("(n p) d -> n p d", p=128)
    wv_r = wvT.rearrange("(n p) d -> n p d", p=128)
    wo_r = woST.rearrange("(n p) e -> n p e", p=128)
    out_r = out.rearrange("(n p) s -> n p s", p=128)

    with tile.TileContext(nc) as tc, nc.allow_low_precision(
            reason="bf16 matmul operands; fp32 PSUM accumulation throughout"):
        _emit(tc, nc, dict(
            xT_r=xT_r, wkq_r=wkq_r, wv_r=wv_r, wo_r=wo_r, out_r=out_r,
            bq2=bq2, bk2=bk2, bvbc=bvbc, cos2=cos2, sin2=sin2,
        ))
    nc.compile()
    return nc


def _emit(tc, nc, d):
    from contextlib import ExitStack
    ctx = ExitStack()
    with ctx:
        consts = ctx.enter_context(tc.tile_pool(name="consts", bufs=1))
        pxq = ctx.enter_context(tc.tile_pool(name="pxq", bufs=16))
        pxh = ctx.enter_context(tc.tile_pool(name="pxh", bufs=8))
        pw = ctx.enter_context(tc.tile_pool(name="pw", bufs=8))
        pwv = ctx.enter_context(tc.tile_pool(name="pwv", bufs=8))
        pwo = ctx.enter_context(tc.tile_pool(name="pwo", bufs=2))
        pqt = ctx.enter_context(tc.tile_pool(name="pqt", bufs=8))
        pkt = ctx.enter_context(tc.tile_pool(name="pkt", bufs=8))
        pv = ctx.enter_context(tc.tile_pool(name="pv", bufs=16))
        pat = ctx.enter_context(tc.tile_pool(name="pat", bufs=8))
        ptmp = ctx.enter_context(tc.tile_pool(name="ptmp", bufs=6))
        pvf_ = ctx.enter_context(tc.tile_pool(name="pvf", bufs=4))
        pbc = ctx.enter_context(tc.tile_pool(name="pbc", bufs=4))
        pstg = ctx.enter_context(tc.tile_pool(name="pstg", bufs=8))
        pe_ = ctx.enter_context(tc.tile_pool(name="pe", bufs=10))
        prec = ctx.enter_context(tc.tile_pool(name="prec", bufs=4))
        psc = ctx.enter_context(tc.tile_pool(name="psc", bufs=2, space="PSUM"))
        ppv = ctx.enter_context(tc.tile_pool(name="ppv", bufs=2, space="PSUM"))
        pbg = ctx.enter_context(tc.tile_pool(name="pbg", bufs=2, space="PSUM"))

        # ---- input DMAs, split across the sync and scalar HWDGE queues.
        # scalar (idle until the first exp) carries ONLY the chunk-loop
        # prefix (wk|wq) + rope tables; sync carries x and everything else.
        # Order within each queue = need order.
        w_sb, wv_sb, wo_sb = {}, {}, []
        xq_sb, xh_sb = {}, {}

        for e in range(NE):
            t = pw.tile([128, 2 * DH], BF16, tag="w")
            nc.scalar.dma_start(t, d["wkq_r"][e])
            w_sb[e] = t
        cos2_sb = consts.tile([128, S], BF16)
        sin2_sb = consts.tile([128, S], BF16)
        nc.scalar.dma_start(cos2_sb[:, 0:1024], d["cos2"][:, 0:1024])
        nc.scalar.dma_start(sin2_sb[:, 0:1024], d["sin2"][:, 0:1024])

        for e in range(NE):
            t = pxq.tile([128, 512], BF16, tag="xq")
            nc.sync.dma_start(t, d["xT_r"][e][:, 0:512])
            xq_sb[(e, 0)] = t
        bq2_sb = consts.tile([128, 2], F32)
        nc.sync.dma_start(bq2_sb, d["bq2"])
        bk2_sb = consts.tile([128, 2], F32)
        nc.sync.dma_start(bk2_sb, d["bk2"])
        for e in range(NE):
            t = pwv.tile([128, DH], BF16, tag="wv")
            nc.sync.dma_start(t, d["wv_r"][e])
            wv_sb[e] = t
        bvbc_sb = consts.tile([128, DH], F32)
        nc.sync.dma_start(bvbc_sb, d["bvbc"])
        for e in range(NE):
            t = pxq.tile([128, 512], BF16, tag="xq")
            nc.sync.dma_start(t, d["xT_r"][e][:, 512:1024])
            xq_sb[(e, 1)] = t
        nc.sync.dma_start(cos2_sb[:, 1024:2048], d["cos2"][:, 1024:2048])
        nc.sync.dma_start(sin2_sb[:, 1024:2048], d["sin2"][:, 1024:2048])
        for e in range(NE):
            t = pxh.tile([128, 1024], BF16, tag="xh")
            nc.sync.dma_start(t, d["xT_r"][e][:, 1024:2048])
            xh_sb[e] = t
        for p in range(2):
            t = pwo.tile([128, E], BF16, tag="wo")
            nc.sync.dma_start(t, d["wo_r"][p])
            wo_sb.append(t)

        warm_sb = consts.tile([128, 512], BF16)
        nc.vector.memset(warm_sb, 0.25)

        def xs(e, scol, w=512):
            """SBUF view of x columns [scol, scol+w) for e-chunk e."""
            if scol < 1024:
                q, off = divmod(scol, 512)
                return xq_sb[(e, q)][:, off:off + w]
            off = scol - 1024
            return xh_sb[e][:, off:off + w]

        # ---- emission: a fine-grained interleave. The PE is the busiest
        # engine mid-kernel; the softmax exps on the scalar engine pace the
        # attention stream. All non-attention PE work is drip-fed between
        # key-blocks; per-block ensures flush producers just in time.
        qt_tiles, kt_tiles, at_tiles = {}, {}, {}
        v_sb = {}
        op_stage = {}

        def rope_tail(ps, bias_sb, dst_pool, dst_tag, tiles, p, idx):
            """Evict a [128,512] qk PSUM accumulator and apply RoPE."""
            cols = slice(idx * 512, idx * 512 + 512)
            tq = ptmp.tile([128, 512], BF16, tag="tmpb")
            nc.vector.tensor_scalar_add(tq, ps, bias_sb[:, p:p + 1])
            tsh = ptmp.tile([128, 512], BF16, tag="tmpb")
            nc.vector.stream_shuffle(tsh, tq, _SWAP_MASK)
            nc.vector.tensor_mul(tsh, tsh, sin2_sb[:, cols])
            nc.vector.tensor_mul(tq, tq, cos2_sb[:, cols])
            qt = dst_pool.tile([128, 512], BF16, tag=dst_tag)
            nc.vector.tensor_add(qt, tq, tsh)
            tiles[(p, idx)] = qt

        def emit_qk_half(w_off, bias_sb, dst_pool, dst_tag, tiles, p, idx):
            """One [128,512] k or q half-tile: 8 e-chunk matmuls + RoPE.
            Yields background units (per e-chunk, then the tail)."""
            ps = pbg.tile([128, 512], F32, tag="bg")
            for e in range(NE):
                def unit(e=e):
                    nc.tensor.matmul(
                        ps,
                        w_sb[e][:, w_off + p * 128:w_off + (p + 1) * 128],
                        xs(e, idx * 512),
                        start=(e == 0), stop=(e == NE - 1),
                    )
                yield 0.22, unit
            yield 0.1, lambda: rope_tail(
                ps, bias_sb, dst_pool, dst_tag, tiles, p, idx)

        def emit_v_unit(st):
            def unit():
                psv = pbg.tile([128, DH], F32, tag="bg")
                for e in range(NE):
                    nc.tensor.matmul(
                        psv,
                        xs(e, st * 128, 128),
                        wv_sb[e],
                        start=(e == 0), stop=(e == NE - 1),
                    )
                vt = pv.tile([128, HPG, 65], BF16, tag="v")
                nc.vector.memset(vt[:, :, 64:65], 1.0)
                nc.vector.tensor_add(
                    vt[:, :, 0:64],
                    psv.rearrange("p (h dd) -> p h dd", dd=64),
                    bvbc_sb.rearrange("p (h dd) -> p h dd", dd=64),
                )
                v_sb[st] = vt
            yield 1.1, unit

        def emit_at_unit(p, j, pvf, rec):
            """Normalize a finished phase: gpsimd-broadcast 1/den across
            partitions, then the two DVE muls. Runs as background (at least
            one phase later) so the reciprocal is long since ready and the
            next phase's exp/affine stream never waits on it."""
            def unit():
                bcsA = pbc.tile([64, 512], F32, tag="bc")
                bcsB = pbc.tile([64, 512], F32, tag="bc")
                nc.gpsimd.partition_broadcast(bcsA, rec[:, 0:512])
                nc.gpsimd.partition_broadcast(bcsB, rec[:, 512:1024])
                at = pat.tile([128, 512], BF16, tag="at")
                nc.vector.tensor_mul(at[0:64], pvf[0:64, 0:512], bcsA)
                nc.vector.tensor_mul(at[64:128], pvf[0:64, 512:1024], bcsB)
                at_tiles[(p, j)] = at
            yield 0.5, unit

        def emit_op_unit(j):
            """Fused output-projection unit: both head-pairs accumulate in
            PSUM, one bf16 staging copy, one DMA."""
            for et_i in range(NE):
                def unit(et_i=et_i):
                    pso = pbg.tile([128, 512], F32, tag="bg")
                    for p in range(2):
                        nc.tensor.matmul(
                            pso,
                            wo_sb[p][:, et_i * 128:(et_i + 1) * 128],
                            at_tiles[(p, j)],
                            start=(p == 0), stop=(p == 1),
                        )
                    ob = pstg.tile([128, 512], BF16, tag="ob")
                    nc.vector.tensor_copy(ob, pso)
                    nc.sync.dma_start(
                        d["out_r"][et_i][:, j * 512:(j + 1) * 512], ob)
                yield 0.7, unit

        def emit_op_p0(j):
            """First head-pair's half of the output projection for q-slice j;
            accumulates into an SBUF stage so it can run as soon as at(0,j)
            exists, one attention phase before at(1,j). Used for the final
            q-slice only, to shorten the tail."""
            for et_i in range(NE):
                def unit(et_i=et_i):
                    pso = pbg.tile([128, 512], F32, tag="bg")
                    nc.tensor.matmul(
                        pso,
                        wo_sb[0][:, et_i * 128:(et_i + 1) * 128],
                        at_tiles[(0, j)],
                        start=True, stop=True,
                    )
                    stg = pstg.tile([128, 512], F32, tag="stg")
                    nc.vector.tensor_copy(stg, pso)
                    op_stage[(j, et_i)] = stg
                yield 0.4, unit

        def emit_op_p1(j):
            for et_i in range(NE):
                def unit(et_i=et_i):
                    pool, tg = (pbg, "bg") if et_i % 2 == 0 else (ppv, "ppv")
                    pso = pool.tile([128, 512], F32, tag=tg)
                    nc.tensor.matmul(
                        pso,
                        wo_sb[1][:, et_i * 128:(et_i + 1) * 128],
                        at_tiles[(1, j)],
                        start=True, stop=True,
                    )
                    ob = pstg.tile([128, 512], BF16, tag="ob")
                    nc.vector.tensor_add(ob, op_stage[(j, et_i)], pso)
                    eng = nc.scalar if et_i % 2 == 0 else nc.sync
                    eng.dma_start(
                        d["out_r"][et_i][:, j * 512:(j + 1) * 512], ob)
                yield 0.45, unit

        # background unit queue + driver, with named completion points
        bg_units = []
        bg_pos = [0]
        kt_ready, qt_ready, v_ready = {}, {}, {}

        def bg_add(gen):
            bg_units.extend(gen)
            return len(bg_units)

        def bg_flush_until(idx):
            while bg_pos[0] < idx:
                bg_units[bg_pos[0]][1]()
                bg_pos[0] += 1

        def bg_take(budget):
            while budget > 0 and bg_pos[0] < len(bg_units):
                cost, fn = bg_units[bg_pos[0]]
                fn()
                bg_pos[0] += 1
                budget -= cost

        def emit_attn(p, j, take=0.55):
            if (p, j) in qt_ready:
                bg_flush_until(qt_ready[(p, j)])
            pvA = ppv.tile([128, 512], F32, tag="ppv")
            pvB = ppv.tile([128, 512], F32, tag="ppv")
            nkb = 4 * j + 4
            for kb in range(nkb):
                s4 = kb // 4
                if (p, s4) in kt_ready:
                    bg_flush_until(kt_ready[(p, s4)])
                m = kb - 4 * j
                c0 = 128 * m if m > 0 else 0
                kt = kt_tiles[(p, s4)]
                kcols = slice((kb % 4) * 128, (kb % 4) * 128 + 128)
                qt = qt_tiles[(p, j)]
                qcols = slice(c0, 512)
                sc = psc.tile([128, 2, 512], F32, tag="sc")
                nc.tensor.matmul(
                    sc[:, 0, c0:512],
                    kt[0:64, kcols],
                    qt[0:64, qcols],
                    start=True, stop=True, tile_position=(0, 0),
                )
                nc.tensor.matmul(
                    sc[:, 1, c0:512],
                    kt[64:128, kcols],
                    qt[64:128, qcols],
                    start=True, stop=True, tile_position=(64, 0),
                )
                et = pe_.tile([128, 2, 512], BF16, tag="e")
                nc.scalar.activation(
                    et[:, :, c0:512], sc[:, :, c0:512], AF.Exp, scale=0.125)
                # ensure this block's V tile producers are emitted before
                # its AV consumers (the flush runs while the exp executes)
                if kb in v_ready:
                    bg_flush_until(v_ready[kb])
                if m >= 0:
                    nc.gpsimd.affine_select(
                        out=et[:, :, c0:c0 + 128],
                        in_=et[:, :, c0:c0 + 128],
                        compare_op=mybir.AluOpType.is_ge,
                        fill=0.0,
                        base=0,
                        pattern=[[0, 2], [1, 128]],
                        channel_multiplier=-1,
                    )
                hA, hB = 2 * p, 2 * p + 1
                nc.tensor.matmul(
                    pvA[0:65, c0:512], v_sb[kb][:, hA, :], et[:, 0, c0:512],
                    start=(kb == 0), stop=(kb == nkb - 1),
                )
                nc.tensor.matmul(
                    pvB[0:65, c0:512], v_sb[kb][:, hB, :], et[:, 1, c0:512],
                    start=(kb == 0), stop=(kb == nkb - 1),
                )
                bg_take(take)
            # Evict the PSUM accumulators to SBUF right away: frees both ppv
            # banks for the next phase's AV matmuls. Phase-end DVE chain is
            # just evict + den + reciprocal; normalization muls run later as
            # a background at-unit.
            pvf = pvf_.tile([128, 1024], F32, tag="pvf")
            nc.vector.tensor_copy(pvf[0:65, 0:512], pvA[0:65, :])
            nc.vector.tensor_copy(pvf[0:65, 512:1024], pvB[0:65, :])
            # den must be a base-0 AP: custom-DVE ops (reciprocal) ignore the
            # input's base partition, so slice pvf[64:65] via a copy first.
            den = prec.tile([1, 1024], F32, tag="den")
            nc.vector.tensor_copy(den, pvf[64:65, :])
            rec = prec.tile([1, 1024], F32, tag="rec")
            nc.vector.reciprocal_approx_fast(rec, den)
            return pvf, rec

        # ---- schedule ----
        # Warm-up: ~3.5us of dummy matmuls on a memset const during the DMA
        # window, so the HAM clock gate reaches K=8/8 (2.4 GHz) before the
        # first real projection instead of running the whole chunk loop at
        # the cold 1.2 GHz default.
        ps_warm = pbg.tile([128, 512], F32, tag="bg")
        for _ in range(10):
            nc.tensor.matmul(ps_warm, warm_sb[:, 0:128], warm_sb,
                             start=True, stop=True)
        # Startup: all four first half-tiles (k00a/q00a/k10a/q10a)
        # interleaved per e-chunk (tracks the xQ0 DMA stream), then RoPE
        # chains on the DVE; v0-3 ensured inside the first phase.
        ps_k00 = pbg.tile([128, 512], F32, tag="bg")
        ps_q00 = pbg.tile([128, 512], F32, tag="bg")
        ps_k10 = ppv.tile([128, 512], F32, tag="ppv")
        ps_q10 = ppv.tile([128, 512], F32, tag="ppv")
        for e in range(NE):
            nc.tensor.matmul(ps_k00, w_sb[e][:, WK0:WK0 + 128], xs(e, 0),
                             start=(e == 0), stop=(e == NE - 1))
            nc.tensor.matmul(ps_q00, w_sb[e][:, WQ0:WQ0 + 128], xs(e, 0),
                             start=(e == 0), stop=(e == NE - 1))
            nc.tensor.matmul(ps_k10, w_sb[e][:, WK0 + 128:WK0 + 256], xs(e, 0),
                             start=(e == 0), stop=(e == NE - 1))
            nc.tensor.matmul(ps_q10, w_sb[e][:, WQ0 + 128:WQ0 + 256], xs(e, 0),
                             start=(e == 0), stop=(e == NE - 1))
        rope_tail(ps_k00, bk2_sb, pkt, "kt", kt_tiles, 0, 0)
        rope_tail(ps_q00, bq2_sb, pqt, "qt", qt_tiles, 0, 0)
        tq_k10 = ptmp.tile([128, 512], BF16, tag="tmpb")
        nc.vector.tensor_scalar_add(tq_k10, ps_k10, bk2_sb[:, 1:2])
        tq_q10 = ptmp.tile([128, 512], BF16, tag="tmpb")
        nc.vector.tensor_scalar_add(tq_q10, ps_q10, bq2_sb[:, 1:2])
        for tq_, pool_, tag_, tiles_ in ((tq_k10, pkt, "kt", kt_tiles),
                                         (tq_q10, pqt, "qt", qt_tiles)):
            tsh_ = ptmp.tile([128, 512], BF16, tag="tmpb")
            nc.vector.stream_shuffle(tsh_, tq_, _SWAP_MASK)
            nc.vector.tensor_mul(tsh_, tsh_, sin2_sb[:, 0:512])
            nc.vector.tensor_mul(tq_, tq_, cos2_sb[:, 0:512])
            qt_ = pool_.tile([128, 512], BF16, tag=tag_)
            nc.vector.tensor_add(qt_, tq_, tsh_)
            tiles_[(1, 0)] = qt_
        v_ready[0] = bg_add(emit_v_unit(0))
        v_ready[1] = bg_add(emit_v_unit(1))
        v_ready[2] = bg_add(emit_v_unit(2))
        v_ready[3] = bg_add(emit_v_unit(3))

        qt_ready[(0, 1)] = bg_add(
            emit_qk_half(WQ0, bq2_sb, pqt, "qt", qt_tiles, 0, 1))
        kt_ready[(0, 1)] = bg_add(
            emit_qk_half(WK0, bk2_sb, pkt, "kt", kt_tiles, 0, 1))
        v_ready[4] = bg_add(emit_v_unit(4))
        v_ready[5] = bg_add(emit_v_unit(5))
        v_ready[6] = bg_add(emit_v_unit(6))
        v_ready[7] = bg_add(emit_v_unit(7))
        qt_ready[(1, 1)] = bg_add(
            emit_qk_half(WQ0, bq2_sb, pqt, "qt", qt_tiles, 1, 1))
        kt_ready[(1, 1)] = bg_add(
            emit_qk_half(WK0, bk2_sb, pkt, "kt", kt_tiles, 1, 1))

        pvf00, rec00 = emit_attn(0, 0)
        bg_add(emit_at_unit(0, 0, pvf00, rec00))
        pvf10, rec10 = emit_attn(1, 0)
        bg_add(emit_at_unit(1, 0, pvf10, rec10))
        pvf01, rec01 = emit_attn(0, 1)
        bg_add(emit_at_unit(0, 1, pvf01, rec01))
        qt_ready[(0, 2)] = bg_add(
            emit_qk_half(WQ0, bq2_sb, pqt, "qt", qt_tiles, 0, 2))
        kt_ready[(0, 2)] = bg_add(
            emit_qk_half(WK0, bk2_sb, pkt, "kt", kt_tiles, 0, 2))
        bg_add(emit_op_unit(0))
        v_ready[8] = bg_add(emit_v_unit(8))
        v_ready[9] = bg_add(emit_v_unit(9))
        v_ready[10] = bg_add(emit_v_unit(10))
        v_ready[11] = bg_add(emit_v_unit(11))
        qt_ready[(1, 2)] = bg_add(
            emit_qk_half(WQ0, bq2_sb, pqt, "qt", qt_tiles, 1, 2))
        kt_ready[(1, 2)] = bg_add(
            emit_qk_half(WK0, bk2_sb, pkt, "kt", kt_tiles, 1, 2))
        pvf11, rec11 = emit_attn(1, 1)
        bg_add(emit_at_unit(1, 1, pvf11, rec11))
        qt_ready[(0, 3)] = bg_add(
            emit_qk_half(WQ0, bq2_sb, pqt, "qt", qt_tiles, 0, 3))
        kt_ready[(0, 3)] = bg_add(
            emit_qk_half(WK0, bk2_sb, pkt, "kt", kt_tiles, 0, 3))
        bg_add(emit_op_unit(1))
        v_ready[12] = bg_add(emit_v_unit(12))
        v_ready[13] = bg_add(emit_v_unit(13))
        v_ready[14] = bg_add(emit_v_unit(14))
        v_ready[15] = bg_add(emit_v_unit(15))
        pvf02, rec02 = emit_attn(0, 2)
        bg_add(emit_at_unit(0, 2, pvf02, rec02))
        qt_ready[(1, 3)] = bg_add(
            emit_qk_half(WQ0, bq2_sb, pqt, "qt", qt_tiles, 1, 3))
        kt_ready[(1, 3)] = bg_add(
            emit_qk_half(WK0, bk2_sb, pkt, "kt", kt_tiles, 1, 3))
        pvf12, rec12 = emit_attn(1, 2)
        bg_add(emit_at_unit(1, 2, pvf12, rec12))
        bg_add(emit_op_unit(2))
        pvf03, rec03 = emit_attn(0, 3, take=0.7)
        bg_add(emit_at_unit(0, 3, pvf03, rec03))
        bg_add(emit_op_p0(3))
        pvf13, rec13 = emit_attn(1, 3, take=0.7)
        bg_flush_until(len(bg_units))
        # keep the PE warm (and the HAM un-throttled) across the final
        # normalization chain so the last output-projection matmuls run at
        # full clock
        ps_warm2 = pbg.tile([128, 512], F32, tag="bg")
        for _ in range(24):
            nc.tensor.matmul(ps_warm2, warm_sb[:, 0:128], warm_sb,
                             start=True, stop=True)
        for cost, fn in emit_at_unit(1, 3, pvf13, rec13):
            fn()
        for cost, fn in emit_op_p1(3):
            fn()


def make_host_inputs(x, Wq, bq, Wk, bk, Wv, bv, Wo, bo):
    """Shard + pre-transpose inputs per core. Returns (in_maps, bo)."""
    x = np.asarray(x, np.float32)
    Wq, Wk, Wv, Wo = (np.asarray(w, np.float32) for w in (Wq, Wk, Wv, Wo))
    bq, bk, bv, bo = (np.asarray(b_, np.float32) for b_ in (bq, bk, bv, bo))

    # RoPE tables
    half = D // 2
    inv_freq = 1.0 / (ROPE_BASE ** (np.arange(half, dtype=np.float64) / half))
    pos = np.arange(S, dtype=np.float64)
    sinus = pos[:, None] * inv_freq[None, :]           # [S, 32]
    sin_full = np.repeat(np.sin(sinus), 2, axis=1)     # [S, 64] interleave-dup
    cos_full = np.repeat(np.cos(sinus), 2, axis=1)
    sgn = np.where(np.arange(D) % 2 == 0, -1.0, 1.0)
    cos2 = np.tile(cos_full.T, (2, 1)).astype(ml_dtypes.bfloat16)
    sin2 = np.tile((sin_full * sgn[None, :]).T, (2, 1)).astype(ml_dtypes.bfloat16)

    xT = [np.ascontiguousarray(x[b_].T) for b_ in range(B)]
    in_maps = []
    for c in range(8):
        b_, hg = c // 4, c % 4
        rows = slice(DH * hg, DH * hg + DH)
        bf = ml_dtypes.bfloat16
        wkq = np.concatenate([Wk[rows].T, Wq[rows].T], axis=1)  # [E, 2*DH]
        in_maps.append({
            "xT": xT[b_].astype(bf),
            "wkq": np.ascontiguousarray(wkq).astype(bf),
            "wvT": np.ascontiguousarray(Wv[rows].T).astype(bf),
            "woST": np.ascontiguousarray(Wo[:, rows].T).astype(bf),
            "bq2": np.ascontiguousarray(bq[rows].reshape(2, 128).T),
            "bk2": np.ascontiguousarray(bk[rows].reshape(2, 128).T),
            "bvbc": np.tile(bv[rows][None, :], (128, 1)).astype(np.float32),
            "cos2": cos2,
            "sin2": sin2,
        })
    return in_maps, bo


_NC_CACHE = {}


def get_nc():
    if "nc" not in _NC_CACHE:
        _NC_CACHE["nc"] = build_nc()
    return _NC_CACHE["nc"]


def kernel(**inputs):
    in_maps, bo = make_host_inputs(**inputs)
    nc = get_nc()
    res = run_bass_kernel_spmd(nc, in_maps, core_ids=list(range(8)))
    out = np.zeros((B, S, E), np.float32)
    for c in range(8):
        out[c // 4] += np.asarray(res.results[c]["out"], np.float32).T
    out += bo[None, None, :]
    return out
